# revision 22
# baseline (speedup 1.0000x reference)
"""Trainium2 kernel for the AR(2) normalizing-flow layer.

Math: the reference builds, per channel d, the stationary AR(2) Toeplitz
autocovariance Sigma_d (T x T), factors it L_d = chol(Sigma_d), and applies
z_out[k,m,d,:] = L_d @ z[k,m,d,:] plus a log-det scalar.

For a stationary AR(2) process the Cholesky factor has closed form: row t >= 2
satisfies L[t,:] = a0*L[t-1,:] + a1*L[t-2,:] + sigma*e_t, with the leading 2x2
block the Cholesky of [[g0,g1],[g1,g0]].  The impulse response decays like
r^k with r <= 0.566 for these coefficients, so L is numerically banded with
bandwidth << 128 in fp32 (entries at lag >= 128 are < 1e-30 relative).  The
O(T^2) matmul therefore reduces to a banded (block bi-diagonal) matmul over
128-wide time blocks:

    out_block[bi] = Hlow @ z_block[bi] + Hhigh @ z_block[bi-1]
    out_block[0]  = Hlow @ z_block[0]  + C @ z_block[0]

where Hlow (lower-triangular Toeplitz) and Hhigh (strictly-upper Toeplitz)
tile the circulant P[t,s] = sigma*h[(t-s) mod 128], and C (rank 2: columns
0,1 only) fixes the stationary initialization.  The log-det reduces to
log(c00) + log(c11) + (T-2)*log(sigma) per channel.

Sharding: D=64 channels split over 8 cores (8 each).  Each core streams its
8 MB z-shard through the TensorEngine (time-major tiles, contraction over the
128 partitions), PSUM -> SBUF -> HBM.  The kernel is HBM-bandwidth-bound, so
inputs, outputs and weights ride three different DGE rings (SP / ACT / SWDGE)
and only the dense circulant P (not Hlow+Hhigh+C) is shipped per channel —
the triangular split happens on-device with one mask multiply + subtract.
Host does only: tap construction from alpha (64x2 values), layout transposes
for DMA-friendly tiles, and the log-det scalar.
"""

import numpy as np

_K, _M, _D, _T = 4, 64, 64, 1024
_B = 128                   # block/band size (partition dim)
_NB = _T // _B             # 8 time blocks
_NCORES = 8
_DLOC = _D // _NCORES      # 8 channels per core
_N = _K * _M               # 256 batch rows
_W = _NB * _N              # 2048 free columns per channel tile

# "float32r" = single-pass fp32 matmul (1 cycle/row at free dim >= 256,
# ~1e-4 relative error); "float32" = exact 2-pass fp32 (4 cycles/row).
_MM_DTYPE = "float32r"

_CACHE = {}


def _build_filters(alpha, log_sigma):
    """Host-side closed-form Cholesky band construction (fp64 -> fp32).

    Returns PT (D, B, B) = transposed circulant (si, ti), CT (D, 2, B) =
    transposed rank-2 initial-block correction, and the exact log-det.
    """
    a0 = alpha[:, 0].astype(np.float64)
    a1 = alpha[:, 1].astype(np.float64)
    sig = np.exp(log_sigma.astype(np.float64))
    g0 = sig**2 / (1.0 - (a0**2 + a1 * a0**2) / (1.0 - a1) - a1**2)
    g1 = a0 * g0 / (1.0 - a1)
    c00 = np.sqrt(g0)
    c10 = g1 / c00
    c11 = np.sqrt(g0 - g1 * g1 / g0)

    n = 2 * _B
    h = np.zeros((_D, n))
    p = np.zeros((_D, n))
    q = np.zeros((_D, n))
    h[:, 0] = 1.0
    h[:, 1] = a0
    p[:, 0] = c00
    p[:, 1] = c10
    q[:, 1] = c11
    for t in range(2, n):
        h[:, t] = a0 * h[:, t - 1] + a1 * h[:, t - 2]
        p[:, t] = a0 * p[:, t - 1] + a1 * p[:, t - 2]
        q[:, t] = a0 * q[:, t - 1] + a1 * q[:, t - 2]
    sh = sig[:, None] * h

    ti = np.arange(_B)[:, None]
    si = np.arange(_B)[None, :]
    lag = ti - si
    Hlow = np.where(lag >= 0, sh[:, np.clip(lag, 0, n - 1)], 0.0)   # (D,ti,si)
    P = sh[:, lag % _B]                                             # circulant
    C = np.zeros((_D, _B, 2))
    C[:, :, 0] = p[:, :_B] - Hlow[:, :, 0]
    C[:, :, 1] = q[:, :_B] - Hlow[:, :, 1]

    logdet = float(np.sum(np.log(c00) + np.log(c11) + (_T - 2) * np.log(sig)))
    PT = np.ascontiguousarray(P.transpose(0, 2, 1))      # (D, si, ti)
    CT = np.ascontiguousarray(C.transpose(0, 2, 1))      # (D, 2, ti)
    return PT.astype(np.float32), CT.astype(np.float32), logdet


def _build_nc(mm_dtype):
    import concourse.bacc as bacc
    import concourse.mybir as mybir
    import concourse.tile as tile

    f32 = mybir.dt.float32
    mmdt = getattr(mybir.dt, mm_dtype)

    nc = bacc.Bacc("TRN2", target_bir_lowering=False, debug=False,
                   num_devices=_NCORES)
    zin = nc.dram_tensor("zin", [_DLOC, _B, _W], mmdt, kind="ExternalInput").ap()
    pt = nc.dram_tensor("pt", [_B, _DLOC * _B], f32, kind="ExternalInput").ap()
    ct = nc.dram_tensor("ct", [2, _DLOC * _B], mmdt, kind="ExternalInput").ap()
    msk = nc.dram_tensor("msk", [_B, 2 * _B], f32, kind="ExternalInput").ap()
    zout = nc.dram_tensor("zout", [_DLOC, _B, _W], f32, kind="ExternalOutput").ap()

    with tile.TileContext(nc) as tc:
        with tc.tile_pool(name="w", bufs=1) as wp, \
             tc.tile_pool(name="z", bufs=8, space="SBUF") as zp, \
             tc.tile_pool(name="o", bufs=4) as op, \
             tc.tile_pool(name="lh", bufs=1) as lhp, \
             tc.tile_pool(name="ps", bufs=8, space="PSUM") as pp:
            # Weights ride the head of the SP ring, before the z stream —
            # the first matmuls gate on them.  (The ACT ring's first
            # transfer starts ~3us late; SWDGE pays a ~10us Q7 drain.)
            ptile = wp.tile([_B, _DLOC * _B], f32)
            nc.sync.dma_start(ptile[:], pt[:])
            mtile = wp.tile([_B, 2 * _B], f32)
            nc.sync.dma_start(mtile[:], msk[:])
            ctile = wp.tile([2, _DLOC * _B], mmdt)
            nc.sync.dma_start(ctile[:], ct[:])

            # Triangular split of the circulant for ALL channels upfront:
            #   wl = P^T . mask(ti>=si),  wh = P^T . mask(ti<si)
            wlhs = lhp.tile([_B, _DLOC * 2 * _B], mmdt)
            for d in range(_DLOC):
                ptd = ptile[:, d * _B:(d + 1) * _B]
                nc.vector.tensor_mul(wlhs[:, (2 * d) * _B:(2 * d + 1) * _B],
                                     ptd, mtile[:, 0:_B])
                nc.vector.tensor_mul(wlhs[:, (2 * d + 1) * _B:(2 * d + 2) * _B],
                                     ptd, mtile[:, _B:2 * _B])

            for d in range(_DLOC):
                zt = zp.tile([_B, _W], mmdt)
                if d == 0:
                    # Finer split at the pipeline head: the first matmuls
                    # only need the first bank's columns.
                    nc.sync.dma_start(zt[:, 0:512], zin[d, :, 0:512])
                    nc.sync.dma_start(zt[:, 512:1024], zin[d, :, 512:1024])
                    nc.sync.dma_start(zt[:, 1024:_W], zin[d, :, 1024:_W])
                else:
                    nc.sync.dma_start(zt[:], zin[d])
                wl = wlhs[:, (2 * d) * _B:(2 * d + 1) * _B]
                wh = wlhs[:, (2 * d + 1) * _B:(2 * d + 2) * _B]
                wc = ctile[:, d * _B:(d + 1) * _B]
                ot = op.tile([_B, _W], f32)
                for b in range(_NB // 2):
                    c0 = 512 * b
                    # One accumulation group per 512-col PSUM bank.  The
                    # leading Hlow matmul covers the whole bank (start=True),
                    # so later accumulating matmuls are order-free on HW;
                    # WAW overlap with the leader pins them after it.
                    ps = pp.tile([_B, 512], f32)
                    nc.tensor.matmul(ps[:], wl, zt[:, c0:c0 + 512],
                                     start=True, stop=False)
                    if b == 0:
                        # block0 has no predecessor: rank-2 init correction
                        # on cols 0:256, Hhigh(z_b0) on cols 256:512.
                        nc.tensor.matmul(ps[:, 0:_N], wc, zt[0:2, 0:_N],
                                         start=False, stop=False,
                                         skip_group_check=True)
                        nc.tensor.matmul(ps[:, _N:512], wh, zt[:, 0:_N],
                                         start=False, stop=True)
                    else:
                        nc.tensor.matmul(ps[:], wh, zt[:, c0 - _N:c0 + _N],
                                         start=False, stop=True)
                    if b % 2 == 0:
                        nc.vector.tensor_copy(ot[:, c0:c0 + 512], ps[:])
                    else:
                        nc.scalar.copy(ot[:, c0:c0 + 512], ps[:])
                    if b == 1:
                        nc.scalar.dma_start(zout[d, :, 0:1024], ot[:, 0:1024])
                # Half-channel output DMAs on the ACT HWDGE ring: the first
                # half leaves while banks 2-3 still compute.
                nc.scalar.dma_start(zout[d, :, 1024:_W], ot[:, 1024:_W])
    nc.finalize()
    return nc


def _get_nc():
    if "nc" not in _CACHE:
        _CACHE["nc"] = _build_nc(_MM_DTYPE)
    return _CACHE["nc"]


def _run(z, sum_log_det_jacobians, alpha, log_sigma, trace=False):
    from concourse.bass_utils import run_bass_kernel_spmd

    z = np.ascontiguousarray(np.asarray(z, dtype=np.float32))
    sldj = np.asarray(sum_log_det_jacobians, dtype=np.float32)
    alpha = np.asarray(alpha, dtype=np.float32)
    log_sigma = np.asarray(log_sigma, dtype=np.float32)

    PT, CT, logdet = _build_filters(alpha, log_sigma)
    lowmask = (np.arange(_B)[None, :] >= np.arange(_B)[:, None])
    mask = np.ascontiguousarray(np.concatenate(
        [lowmask, ~lowmask], axis=1).astype(np.float32)
    )  # mask[si, 0:B] keeps Hlow^T (ti >= si); mask[si, B:2B] keeps Hhigh^T

    # z (K,M,D,T) -> (D, si, bi, n): time-major per channel, 128-partition tiles
    zt = np.ascontiguousarray(
        z.reshape(_N, _D, _NB, _B).transpose(1, 3, 2, 0)
    ).reshape(_D, _B, _W)

    in_maps = []
    for c in range(_NCORES):
        sl = slice(c * _DLOC, (c + 1) * _DLOC)
        in_maps.append({
            "zin": np.ascontiguousarray(zt[sl]),
            "pt": np.ascontiguousarray(
                PT[sl].transpose(1, 0, 2)).reshape(_B, _DLOC * _B),
            "ct": np.ascontiguousarray(
                CT[sl].transpose(1, 0, 2)).reshape(2, _DLOC * _B),
            "msk": mask,
        })

    nc = _get_nc()
    res = run_bass_kernel_spmd(nc, in_maps, core_ids=list(range(_NCORES)),
                               trace=trace)

    out = np.concatenate([r["zout"] for r in res.results], axis=0)  # (D,B,W)
    out = out.reshape(_D, _B, _NB, _N).transpose(3, 0, 2, 1)        # (n,D,bi,ti)
    z_out = np.ascontiguousarray(out).reshape(_K, _M, _D, _T)

    sldj_out = (sldj.astype(np.float64) + logdet).astype(np.float32)
    return (z_out, sldj_out), res


def kernel(z, sum_log_det_jacobians, alpha, log_sigma):
    (z_out, sldj_out), _ = _run(z, sum_log_det_jacobians, alpha, log_sigma)
    return z_out, sldj_out


# revision 25
# speedup vs baseline: 1.0145x; 1.0145x over previous
"""Trainium2 kernel for the AR(2) normalizing-flow layer.

Math: the reference builds, per channel d, the stationary AR(2) Toeplitz
autocovariance Sigma_d (T x T), factors it L_d = chol(Sigma_d), and applies
z_out[k,m,d,:] = L_d @ z[k,m,d,:] plus a log-det scalar.

For a stationary AR(2) process the Cholesky factor has closed form: row t >= 2
satisfies L[t,:] = a0*L[t-1,:] + a1*L[t-2,:] + sigma*e_t, with the leading 2x2
block the Cholesky of [[g0,g1],[g1,g0]].  The impulse response decays like
r^k with r <= 0.566 for these coefficients, so L is numerically banded with
bandwidth << 128 in fp32 (entries at lag >= 128 are < 1e-30 relative).  The
O(T^2) matmul therefore reduces to a banded (block bi-diagonal) matmul over
128-wide time blocks:

    out_block[bi] = Hlow @ z_block[bi] + Hhigh @ z_block[bi-1]
    out_block[0]  = Hlow @ z_block[0]  + C @ z_block[0]

where Hlow (lower-triangular Toeplitz) and Hhigh (strictly-upper Toeplitz)
tile the circulant P[t,s] = sigma*h[(t-s) mod 128], and C (rank 2: columns
0,1 only) fixes the stationary initialization.  The log-det reduces to
log(c00) + log(c11) + (T-2)*log(sigma) per channel.

Sharding: D=64 channels split over 8 cores (8 each).  Each core streams its
8 MB z-shard through the TensorEngine (time-major tiles, contraction over the
128 partitions), PSUM -> SBUF -> HBM.  The kernel is HBM-bandwidth-bound, so
inputs, outputs and weights ride three different DGE rings (SP / ACT / SWDGE)
and only the dense circulant P (not Hlow+Hhigh+C) is shipped per channel —
the triangular split happens on-device with one mask multiply + subtract.
Host does only: tap construction from alpha (64x2 values), layout transposes
for DMA-friendly tiles, and the log-det scalar.
"""

import numpy as np

_K, _M, _D, _T = 4, 64, 64, 1024
_B = 128                   # block/band size (partition dim)
_NB = _T // _B             # 8 time blocks
_NCORES = 8
_DLOC = _D // _NCORES      # 8 channels per core
_N = _K * _M               # 256 batch rows
_W = _NB * _N              # 2048 free columns per channel tile

# "float32r" = single-pass fp32 matmul (1 cycle/row at free dim >= 256,
# ~1e-4 relative error); "float32" = exact 2-pass fp32 (4 cycles/row).
_MM_DTYPE = "float32r"

_CACHE = {}


def _build_filters(alpha, log_sigma):
    """Host-side closed-form Cholesky band construction (fp64 -> fp32).

    Returns PT (D, B, B) = transposed circulant (si, ti), CT (D, 2, B) =
    transposed rank-2 initial-block correction, and the exact log-det.
    """
    a0 = alpha[:, 0].astype(np.float64)
    a1 = alpha[:, 1].astype(np.float64)
    sig = np.exp(log_sigma.astype(np.float64))
    g0 = sig**2 / (1.0 - (a0**2 + a1 * a0**2) / (1.0 - a1) - a1**2)
    g1 = a0 * g0 / (1.0 - a1)
    c00 = np.sqrt(g0)
    c10 = g1 / c00
    c11 = np.sqrt(g0 - g1 * g1 / g0)

    n = 2 * _B
    h = np.zeros((_D, n))
    p = np.zeros((_D, n))
    q = np.zeros((_D, n))
    h[:, 0] = 1.0
    h[:, 1] = a0
    p[:, 0] = c00
    p[:, 1] = c10
    q[:, 1] = c11
    for t in range(2, n):
        h[:, t] = a0 * h[:, t - 1] + a1 * h[:, t - 2]
        p[:, t] = a0 * p[:, t - 1] + a1 * p[:, t - 2]
        q[:, t] = a0 * q[:, t - 1] + a1 * q[:, t - 2]
    sh = sig[:, None] * h

    ti = np.arange(_B)[:, None]
    si = np.arange(_B)[None, :]
    lag = ti - si
    Hlow = np.where(lag >= 0, sh[:, np.clip(lag, 0, n - 1)], 0.0)   # (D,ti,si)
    P = sh[:, lag % _B]                                             # circulant
    C = np.zeros((_D, _B, 2))
    C[:, :, 0] = p[:, :_B] - Hlow[:, :, 0]
    C[:, :, 1] = q[:, :_B] - Hlow[:, :, 1]

    logdet = float(np.sum(np.log(c00) + np.log(c11) + (_T - 2) * np.log(sig)))
    PT = np.ascontiguousarray(P.transpose(0, 2, 1))      # (D, si, ti)
    CT = np.ascontiguousarray(C.transpose(0, 2, 1))      # (D, 2, ti)
    return PT.astype(np.float32), CT.astype(np.float32), logdet


def _build_nc(mm_dtype):
    import concourse.bacc as bacc
    import concourse.mybir as mybir
    import concourse.tile as tile

    f32 = mybir.dt.float32
    mmdt = getattr(mybir.dt, mm_dtype)

    nc = bacc.Bacc("TRN2", target_bir_lowering=False, debug=False,
                   num_devices=_NCORES)
    zin = nc.dram_tensor("zin", [_DLOC, _B, _W], mmdt, kind="ExternalInput").ap()
    pt = nc.dram_tensor("pt", [_B, _DLOC * _B], f32, kind="ExternalInput").ap()
    ct = nc.dram_tensor("ct", [2, _DLOC * _B], mmdt, kind="ExternalInput").ap()
    msk = nc.dram_tensor("msk", [_B, 2 * _B], f32, kind="ExternalInput").ap()
    zout = nc.dram_tensor("zout", [_DLOC, _B, _W], f32, kind="ExternalOutput").ap()

    with tile.TileContext(nc) as tc:
        with tc.tile_pool(name="w", bufs=1) as wp, \
             tc.tile_pool(name="z", bufs=8, space="SBUF") as zp, \
             tc.tile_pool(name="o", bufs=3) as op, \
             tc.tile_pool(name="lh", bufs=3) as lhp, \
             tc.tile_pool(name="ps", bufs=8, space="PSUM") as pp:
            # Small constant-ish inputs ride the ACT HWDGE ring (idle early)
            # so the big z stream on the SP ring starts at t=0.  (SWDGE /
            # gpsimd pays a ~10us Q7 drain — keep it out of the kernel.)
            mtile = wp.tile([_B, 2 * _B], f32)
            nc.scalar.dma_start(mtile[:], msk[:])
            ptile = wp.tile([_B, _DLOC * _B], f32)
            nc.scalar.dma_start(ptile[:], pt[:])
            ctile = wp.tile([2, _DLOC * _B], mmdt)
            nc.scalar.dma_start(ctile[:], ct[:])

            for d in range(_DLOC):
                zt = zp.tile([_B, _W], mmdt)
                nc.sync.dma_start(zt[:], zin[d])
                # Triangular split of the circulant, on-device:
                #   wl = P^T . mask(ti>=si),  wh = P^T . mask(ti<si)
                wlh = lhp.tile([_B, 2 * _B], mmdt)
                wl = wlh[:, 0:_B]
                wh = wlh[:, _B:2 * _B]
                ptd = ptile[:, d * _B:(d + 1) * _B]
                nc.vector.tensor_mul(wl, ptd, mtile[:, 0:_B])
                nc.vector.tensor_mul(wh, ptd, mtile[:, _B:2 * _B])
                wc = ctile[:, d * _B:(d + 1) * _B]
                ot = op.tile([_B, _W], f32)
                for b in range(_NB // 2):
                    c0 = 512 * b
                    # One accumulation group per 512-col PSUM bank.  The
                    # leading Hlow matmul covers the whole bank (start=True),
                    # so later accumulating matmuls are order-free on HW;
                    # WAW overlap with the leader pins them after it.
                    ps = pp.tile([_B, 512], f32)
                    nc.tensor.matmul(ps[:], wl, zt[:, c0:c0 + 512],
                                     start=True, stop=False)
                    if b == 0:
                        # block0 has no predecessor: rank-2 init correction
                        # on cols 0:256, Hhigh(z_b0) on cols 256:512.
                        nc.tensor.matmul(ps[:, 0:_N], wc, zt[0:2, 0:_N],
                                         start=False, stop=False,
                                         skip_group_check=True)
                        nc.tensor.matmul(ps[:, _N:512], wh, zt[:, 0:_N],
                                         start=False, stop=True)
                    else:
                        nc.tensor.matmul(ps[:], wh, zt[:, c0 - _N:c0 + _N],
                                         start=False, stop=True)
                    if b % 2 == 0:
                        nc.vector.tensor_copy(ot[:, c0:c0 + 512], ps[:])
                    else:
                        nc.scalar.copy(ot[:, c0:c0 + 512], ps[:])
                # One output DMA per channel on the ACT HWDGE ring — each
                # trigger blocks its engine ~0.6us, so fewer is better.
                nc.scalar.dma_start(zout[d], ot[:])
    nc.finalize()
    return nc


def _get_nc():
    if "nc" not in _CACHE:
        _CACHE["nc"] = _build_nc(_MM_DTYPE)
    return _CACHE["nc"]


def _run(z, sum_log_det_jacobians, alpha, log_sigma, trace=False):
    from concourse.bass_utils import run_bass_kernel_spmd

    z = np.ascontiguousarray(np.asarray(z, dtype=np.float32))
    sldj = np.asarray(sum_log_det_jacobians, dtype=np.float32)
    alpha = np.asarray(alpha, dtype=np.float32)
    log_sigma = np.asarray(log_sigma, dtype=np.float32)

    PT, CT, logdet = _build_filters(alpha, log_sigma)
    lowmask = (np.arange(_B)[None, :] >= np.arange(_B)[:, None])
    mask = np.ascontiguousarray(np.concatenate(
        [lowmask, ~lowmask], axis=1).astype(np.float32)
    )  # mask[si, 0:B] keeps Hlow^T (ti >= si); mask[si, B:2B] keeps Hhigh^T

    # z (K,M,D,T) -> (D, si, bi, n): time-major per channel, 128-partition tiles
    zt = np.ascontiguousarray(
        z.reshape(_N, _D, _NB, _B).transpose(1, 3, 2, 0)
    ).reshape(_D, _B, _W)

    in_maps = []
    for c in range(_NCORES):
        sl = slice(c * _DLOC, (c + 1) * _DLOC)
        in_maps.append({
            "zin": np.ascontiguousarray(zt[sl]),
            "pt": np.ascontiguousarray(
                PT[sl].transpose(1, 0, 2)).reshape(_B, _DLOC * _B),
            "ct": np.ascontiguousarray(
                CT[sl].transpose(1, 0, 2)).reshape(2, _DLOC * _B),
            "msk": mask,
        })

    nc = _get_nc()
    res = run_bass_kernel_spmd(nc, in_maps, core_ids=list(range(_NCORES)),
                               trace=trace)

    out = np.concatenate([r["zout"] for r in res.results], axis=0)  # (D,B,W)
    out = out.reshape(_D, _B, _NB, _N).transpose(3, 0, 2, 1)        # (n,D,bi,ti)
    z_out = np.ascontiguousarray(out).reshape(_K, _M, _D, _T)

    sldj_out = (sldj.astype(np.float64) + logdet).astype(np.float32)
    return (z_out, sldj_out), res


def kernel(z, sum_log_det_jacobians, alpha, log_sigma):
    (z_out, sldj_out), _ = _run(z, sum_log_det_jacobians, alpha, log_sigma)
    return z_out, sldj_out


# revision 26
# speedup vs baseline: 1.0509x; 1.0358x over previous
"""Trainium2 kernel for the AR(2) normalizing-flow layer.

Math: the reference builds, per channel d, the stationary AR(2) Toeplitz
autocovariance Sigma_d (T x T), factors it L_d = chol(Sigma_d), and applies
z_out[k,m,d,:] = L_d @ z[k,m,d,:] plus a log-det scalar.

For a stationary AR(2) process the Cholesky factor has closed form: row t >= 2
satisfies L[t,:] = a0*L[t-1,:] + a1*L[t-2,:] + sigma*e_t, with the leading 2x2
block the Cholesky of [[g0,g1],[g1,g0]].  The impulse response decays like
r^k with r <= 0.566 for these coefficients, so L is numerically banded with
bandwidth << 128 in fp32 (entries at lag >= 128 are < 1e-30 relative).  The
O(T^2) matmul therefore reduces to a banded (block bi-diagonal) matmul over
128-wide time blocks:

    out_block[bi] = Hlow @ z_block[bi] + Hhigh @ z_block[bi-1]
    out_block[0]  = Hlow @ z_block[0]  + C @ z_block[0]

where Hlow (lower-triangular Toeplitz) and Hhigh (strictly-upper Toeplitz)
tile the circulant P[t,s] = sigma*h[(t-s) mod 128], and C (rank 2: columns
0,1 only) fixes the stationary initialization.  The log-det reduces to
log(c00) + log(c11) + (T-2)*log(sigma) per channel.

Sharding: D=64 channels split over 8 cores (8 each).  Each core streams its
8 MB z-shard through the TensorEngine (time-major tiles, contraction over the
128 partitions), PSUM -> SBUF -> HBM.  The kernel is DMA-fabric-bound
(~430 GB/s/core), so inputs ride the SP HWDGE ring while outputs and the
small weights ride the ACT ring, and only the dense circulant P (not
Hlow+Hhigh) is shipped per channel — the triangular split happens on-device
with two DVE mask multiplies.  Host does only: tap construction from alpha
(64x2 values), layout transposes for DMA-friendly tiles, and the log-det
scalar.
"""

import numpy as np

_K, _M, _D, _T = 4, 64, 64, 1024
_B = 128                   # block/band size (partition dim)
_NB = _T // _B             # 8 time blocks
_NCORES = 8
_DLOC = _D // _NCORES      # 8 channels per core
_N = _K * _M               # 256 batch rows
_W = _NB * _N              # 2048 free columns per channel tile

# "float32r" = single-pass fp32 matmul (1 cycle/row at free dim >= 256,
# ~1e-4 relative error); "float32" = exact 2-pass fp32 (4 cycles/row).
_MM_DTYPE = "float32r"

_CACHE = {}


def _build_filters(alpha, log_sigma):
    """Host-side closed-form Cholesky band construction (fp64 -> fp32).

    Returns PT (D, B, B) = transposed circulant (si, ti), CT (D, 2, B) =
    transposed rank-2 initial-block correction, and the exact log-det.
    """
    a0 = alpha[:, 0].astype(np.float64)
    a1 = alpha[:, 1].astype(np.float64)
    sig = np.exp(log_sigma.astype(np.float64))
    g0 = sig**2 / (1.0 - (a0**2 + a1 * a0**2) / (1.0 - a1) - a1**2)
    g1 = a0 * g0 / (1.0 - a1)
    c00 = np.sqrt(g0)
    c10 = g1 / c00
    c11 = np.sqrt(g0 - g1 * g1 / g0)

    n = 2 * _B
    h = np.zeros((_D, n))
    p = np.zeros((_D, n))
    q = np.zeros((_D, n))
    h[:, 0] = 1.0
    h[:, 1] = a0
    p[:, 0] = c00
    p[:, 1] = c10
    q[:, 1] = c11
    for t in range(2, n):
        h[:, t] = a0 * h[:, t - 1] + a1 * h[:, t - 2]
        p[:, t] = a0 * p[:, t - 1] + a1 * p[:, t - 2]
        q[:, t] = a0 * q[:, t - 1] + a1 * q[:, t - 2]
    sh = sig[:, None] * h

    ti = np.arange(_B)[:, None]
    si = np.arange(_B)[None, :]
    lag = ti - si
    Hlow = np.where(lag >= 0, sh[:, np.clip(lag, 0, n - 1)], 0.0)   # (D,ti,si)
    P = sh[:, lag % _B]                                             # circulant
    C = np.zeros((_D, _B, 2))
    C[:, :, 0] = p[:, :_B] - Hlow[:, :, 0]
    C[:, :, 1] = q[:, :_B] - Hlow[:, :, 1]

    logdet = float(np.sum(np.log(c00) + np.log(c11) + (_T - 2) * np.log(sig)))
    PT = np.ascontiguousarray(P.transpose(0, 2, 1))      # (D, si, ti)
    CT = np.ascontiguousarray(C.transpose(0, 2, 1))      # (D, 2, ti)
    return PT.astype(np.float32), CT.astype(np.float32), logdet


def _build_nc(mm_dtype):
    import concourse.bacc as bacc
    import concourse.mybir as mybir
    import concourse.tile as tile

    f32 = mybir.dt.float32
    mmdt = getattr(mybir.dt, mm_dtype)

    nc = bacc.Bacc("TRN2", target_bir_lowering=False, debug=False,
                   num_devices=_NCORES)
    zin = nc.dram_tensor("zin", [_DLOC, _B, _W], mmdt, kind="ExternalInput").ap()
    pt = nc.dram_tensor("pt", [_B, _DLOC * _B], f32, kind="ExternalInput").ap()
    ct = nc.dram_tensor("ct", [2, _DLOC * _B], mmdt, kind="ExternalInput").ap()
    msk = nc.dram_tensor("msk", [_B, 2 * _B], f32, kind="ExternalInput").ap()
    zout = nc.dram_tensor("zout", [_DLOC, _B, _W], f32, kind="ExternalOutput").ap()

    with tile.TileContext(nc) as tc:
        with tc.tile_pool(name="w", bufs=1) as wp, \
             tc.tile_pool(name="z", bufs=8, space="SBUF") as zp, \
             tc.tile_pool(name="o", bufs=3) as op, \
             tc.tile_pool(name="lh", bufs=3) as lhp, \
             tc.tile_pool(name="ps", bufs=8, space="PSUM") as pp:
            # Small constant-ish inputs ride the ACT HWDGE ring (idle early)
            # so the big z stream on the SP ring starts at t=0.  (SWDGE /
            # gpsimd pays a ~10us Q7 drain — keep it out of the kernel.)
            mtile = wp.tile([_B, 2 * _B], f32)
            nc.scalar.dma_start(mtile[:], msk[:])
            ptile = wp.tile([_B, _DLOC * _B], f32)
            nc.scalar.dma_start(ptile[:], pt[:])
            ctile = wp.tile([2, _DLOC * _B], mmdt)
            nc.scalar.dma_start(ctile[:], ct[:])

            for d in range(_DLOC):
                zt = zp.tile([_B, _W], mmdt)
                nc.sync.dma_start(zt[:], zin[d])
                # Triangular split of the circulant, on-device:
                #   wl = P^T . mask(ti>=si),  wh = P^T . mask(ti<si)
                wlh = lhp.tile([_B, 2 * _B], mmdt)
                wl = wlh[:, 0:_B]
                wh = wlh[:, _B:2 * _B]
                ptd = ptile[:, d * _B:(d + 1) * _B]
                nc.vector.tensor_mul(wl, ptd, mtile[:, 0:_B])
                nc.vector.tensor_mul(wh, ptd, mtile[:, _B:2 * _B])
                wc = ctile[:, d * _B:(d + 1) * _B]
                ot = op.tile([_B, _W], f32)
                for b in range(_NB // 2):
                    c0 = 512 * b
                    # One accumulation group per 512-col PSUM bank.  The
                    # leading Hlow matmul covers the whole bank (start=True),
                    # so later accumulating matmuls are order-free on HW;
                    # WAW overlap with the leader pins them after it.
                    ps = pp.tile([_B, 512], f32)
                    nc.tensor.matmul(ps[:], wl, zt[:, c0:c0 + 512],
                                     start=True, stop=False)
                    if b == 0:
                        # block0 has no predecessor: rank-2 init correction
                        # on cols 0:256, Hhigh(z_b0) on cols 256:512.
                        nc.tensor.matmul(ps[:, 0:_N], wc, zt[0:2, 0:_N],
                                         start=False, stop=False,
                                         skip_group_check=True)
                        nc.tensor.matmul(ps[:, _N:512], wh, zt[:, 0:_N],
                                         start=False, stop=True)
                    else:
                        nc.tensor.matmul(ps[:], wh, zt[:, c0 - _N:c0 + _N],
                                         start=False, stop=True)
                    if b % 2 == 0:
                        nc.vector.tensor_copy(ot[:, c0:c0 + 512], ps[:])
                    else:
                        nc.scalar.copy(ot[:, c0:c0 + 512], ps[:])
                # One output DMA per channel on the ACT HWDGE ring — each
                # trigger blocks its engine ~0.6us, so fewer is better.
                nc.scalar.dma_start(zout[d], ot[:])
    nc.finalize()
    return nc


def _get_nc():
    if "nc" not in _CACHE:
        _CACHE["nc"] = _build_nc(_MM_DTYPE)
    return _CACHE["nc"]


def _run(z, sum_log_det_jacobians, alpha, log_sigma, trace=False):
    from concourse.bass_utils import run_bass_kernel_spmd

    z = np.ascontiguousarray(np.asarray(z, dtype=np.float32))
    sldj = np.asarray(sum_log_det_jacobians, dtype=np.float32)
    alpha = np.asarray(alpha, dtype=np.float32)
    log_sigma = np.asarray(log_sigma, dtype=np.float32)

    PT, CT, logdet = _build_filters(alpha, log_sigma)
    lowmask = (np.arange(_B)[None, :] >= np.arange(_B)[:, None])
    mask = np.ascontiguousarray(np.concatenate(
        [lowmask, ~lowmask], axis=1).astype(np.float32)
    )  # mask[si, 0:B] keeps Hlow^T (ti >= si); mask[si, B:2B] keeps Hhigh^T

    # z (K,M,D,T) -> (D, si, bi, n): time-major per channel, 128-partition tiles
    zt = np.ascontiguousarray(
        z.reshape(_N, _D, _NB, _B).transpose(1, 3, 2, 0)
    ).reshape(_D, _B, _W)

    in_maps = []
    for c in range(_NCORES):
        sl = slice(c * _DLOC, (c + 1) * _DLOC)
        in_maps.append({
            "zin": np.ascontiguousarray(zt[sl]),
            "pt": np.ascontiguousarray(
                PT[sl].transpose(1, 0, 2)).reshape(_B, _DLOC * _B),
            "ct": np.ascontiguousarray(
                CT[sl].transpose(1, 0, 2)).reshape(2, _DLOC * _B),
            "msk": mask,
        })

    nc = _get_nc()
    res = run_bass_kernel_spmd(nc, in_maps, core_ids=list(range(_NCORES)),
                               trace=trace)

    out = np.concatenate([r["zout"] for r in res.results], axis=0)  # (D,B,W)
    out = out.reshape(_D, _B, _NB, _N).transpose(3, 0, 2, 1)        # (n,D,bi,ti)
    z_out = np.ascontiguousarray(out).reshape(_K, _M, _D, _T)

    sldj_out = (sldj.astype(np.float64) + logdet).astype(np.float32)
    return (z_out, sldj_out), res


def kernel(z, sum_log_det_jacobians, alpha, log_sigma):
    (z_out, sldj_out), _ = _run(z, sum_log_det_jacobians, alpha, log_sigma)
    return z_out, sldj_out
